# revision 49
# baseline (speedup 1.0000x reference)
"""CarafeGuidedUpsampler Trainium2 kernel (v2).

Sharding: 8 cores = 4 samples x 2 vertical halves. Bottom halves are
row-flipped on the host (with a matching encoder-weight permutation) so all
cores run one SPMD program: "compute output rows [0,112) from source rows
[0,9) + guidance".

Per-core device program (fp32 data, matmuls as float32r):
  4x CARAFE stage: compress 1x1 conv (PE) -> encoder 3x3 conv + exp (PE+ACT)
  -> softmax normalize (PE k-sum, DVE recip, PE broadcast matmul, DVE mul) ->
  CARAFE reassembly as banded matmuls: normalized masks are scattered into a
  contiguous 3-slot band image in DRAM (diagonal access patterns are only
  legal on the DRAM side), loaded back as one [K, 3*NB] tile, and contracted
  against column-partition "xS" slots with PSUM accumulation over the 3
  vertical taps.

Stage 4 fuses the final 1x1 projection through the reassembly: slots hold
P^T-projected pixels (ys = proj(x)), so the band matmuls emit projected
output channels directly; the 3 guidance channels enter as K=3 matmuls and
the bias rides on the PSUM->SBUF drain activation. Stage-4 masks use a
pq-major 36-channel layout so the scatter writes 4-element contiguous runs
(4x fewer DMA descriptors).
"""

import numpy as np
from contextlib import ExitStack

import concourse.bass as bass
import concourse.tile as tile
from concourse import bacc, mybir
from concourse.bass_utils import run_bass_kernel_spmd

F32 = mybir.dt.float32
F32R = mybir.dt.float32r
BF16 = mybir.dt.bfloat16

DIM = 384
COMP = 64
CIN = 387

# per-stage geometry (stage index 0..3)
W_IN = [14, 28, 56, 112]          # input width
ROWS_IN = [9, 16, 30, 57]         # input rows incl. halo
MROWS = [8, 15, 29, 56]           # mask rows computed
PP = [8, 4, 2, 1]                 # input rows packed per xS slot
WBLK = [16, 32, 64, 113]          # partition stride per packed row
NB = 448                          # band matmul N = 4*P*W (equal all stages)
KK_ = [PP[s] * WBLK[s] for s in range(4)]   # [128, 128, 128, 113]
OUT_ROWS = [16, 30, 57, 112]      # output rows kept (== ROWS_IN[s+1])
HR, WR = 112, 224
IMG_ROWS = [1 + 3 * KK_[s] for s in range(4)]


def resize_matrix(n_in, n_out):
    """Row-resize matrix matching jax.image.resize(method='bilinear')."""
    scale = n_out / n_in
    j = np.arange(n_in, dtype=np.float64)
    out = np.zeros((n_out, n_in), np.float64)
    for i in range(n_out):
        center = (i + 0.5) / scale - 0.5
        w = np.maximum(0.0, 1.0 - np.abs((j - center) * scale))
        out[i] = w / w.sum()
    return out.astype(np.float32)


def _perm_for_core(flip):
    """perm[ch'] = original encoder out-channel; ch' = di*12+dj*4+p*2+q."""
    perm = np.zeros(36, np.int64)
    for di in range(3):
        for dj in range(3):
            for p in range(2):
                for q in range(2):
                    sdi, sp = (2 - di, 1 - p) if flip else (di, p)
                    perm[di * 12 + dj * 4 + p * 2 + q] = (sdi * 3 + dj) * 4 + sp * 2 + q
    return perm


def _cblob_spec():
    """(name, partitions, cols) of the packed f32 constant blob, in order.
    Constants needed by stages 0/1 come first so the blob can be loaded in
    two DMAs with the early half gating less work."""
    spec = [("ident", 128, 128), ("sel96", 96, 4), ("bc96", 4, 96),
            ("sel36", 128, 4), ("bc36", 4, 128)]
    for s in range(4):
        for c in range(3):
            spec.append((f"cwT{s}_{c}", 128, COMP))
        spec.append((f"cwT{s}_g", 3, COMP))
        nch = 96 if s == 2 else 128
        for t in range(9):
            spec.append((f"ewT{s}_{t}", COMP, nch))
        if s == 1:
            spec += [("pjT0", 128, DIM), ("pjT1", 128, DIM), ("pjT2", 128, DIM),
                     ("pjTb4", 4, DIM)]
    return spec


def _cblob_split():
    """Column where the early/late cblob halves divide (end of s1 weights)."""
    c0 = 0
    for nm, _, c in _cblob_spec():
        c0 += c
        if nm == "ewT1_8":
            return c0
    raise AssertionError


def _rblob_spec():
    """Constants used only by the guidance-resize phase."""
    spec = [("rident", 128, 128)]
    for c in range(3):
        for k in range(2):
            spec.append((f"gT{c}_{k}", 128 if k == 0 else 96, 224))
    for s in range(4):
        for k in range(2):
            spec.append((f"Cm{s}_{k}", 128 if k == 0 else 96, W_IN[s]))
        for k in range(2):
            spec.append((f"RmT{s}_{k}", 128 if k == 0 else 96, ROWS_IN[s]))
    return spec


class _Ref:
    """Column-offset view into a packed blob tile, sliceable like a tile."""

    def __init__(self, tile_, p, c0, cols):
        self.t, self.p, self.c0, self.cols = tile_, p, c0, cols

    def __getitem__(self, idx):
        if not isinstance(idx, tuple):
            idx = (idx, slice(None))
        ps, cs = idx
        p0 = ps.start if ps.start is not None else 0
        p1 = ps.stop if ps.stop is not None else self.p
        c0 = cs.start if cs.start is not None else 0
        c1 = cs.stop if cs.stop is not None else self.cols
        return self.t[p0:p1, self.c0 + c0:self.c0 + c1]


# ----------------------------------------------------------------------------
# device program
# ----------------------------------------------------------------------------

def build_bass(n_stages=4):
    nc = bacc.Bacc("TRN2", target_bir_lowering=False, debug=False,
                   enable_asserts=False, num_devices=8)
    d = {}

    def din(name, shape, dt=F32):
        d[name] = nc.dram_tensor(name, list(shape), dt, kind="ExternalInput").ap()
        return d[name]

    din("src_conv", (DIM, ROWS_IN[0] * W_IN[0]))
    din("xs1", (128, 2, DIM))
    din("guid_hr", (3, HR * WR))
    din("ones1", (1, NB))
    CSPEC, RSPEC = _cblob_spec(), _rblob_spec()
    din("cblob", (128, sum(c for _, _, c in CSPEC)))
    din("rblob", (128, sum(c for _, _, c in RSPEC)))
    for s in range(4):
        nch = 96 if s == 2 else 128
        din(f"cb{s}", (COMP, 1))
        din(f"eb{s}", (nch, 1))
        din(f"imgz{s}", (4, IMG_ROWS[s], NB), dt=(F32R if s <= 1 else BF16))

    if n_stages == 4:
        out = nc.dram_tensor("out", [DIM, HR, WR], BF16, kind="ExternalOutput").ap()
    else:
        s = n_stages - 1
        out = nc.dram_tensor(
            "out", [DIM, OUT_ROWS[s], 2 * W_IN[s]], F32, kind="ExternalOutput").ap()

    g_dram = [nc.dram_tensor(f"gd{s}", [3, ROWS_IN[s] * W_IN[s]], F32).ap()
              for s in range(4)]

    with tile.TileContext(nc) as tc, ExitStack() as top:
        const = top.enter_context(tc.tile_pool(name="const", bufs=1))
        persist = top.enter_context(tc.tile_pool(name="persist", bufs=1))

        def sb_from(dram_ap, shape, name, dt=F32R):
            t = const.tile(list(shape), dt, name=name, tag=name)
            nc.sync.dma_start(t[:], dram_ap.bitcast(dt) if dt is F32R else dram_ap)
            return t

        grc = top.enter_context(tc.tile_pool(name="grc", bufs=1))
        rblob_t = grc.tile([128, sum(c for _, _, c in RSPEC)], F32R,
                           name="rblob", tag="rblob")
        nc.scalar.dma_start(rblob_t[:], d["rblob"].bitcast(F32R))
        cblob_t = const.tile([128, sum(c for _, _, c in CSPEC)], F32R,
                             name="cblob", tag="cblob")
        csplit = _cblob_split()
        nc.sync.dma_start(cblob_t[:, 0:csplit],
                          d["cblob"][:, 0:csplit].bitcast(F32R))
        cref = {}
        cc0 = 0
        for nm, p, c in CSPEC:
            cref[nm] = _Ref(cblob_t, p, cc0, c)
            cc0 += c
        ident, sel96, bc96 = cref["ident"], cref["sel96"], cref["bc96"]
        sel36, bc36 = cref["sel36"], cref["bc36"]
        projT = [cref[f"pjT{c}"] for c in range(3)]
        projTb4 = cref["pjTb4"]
        cwT = [[cref[f"cwT{s}_{c}"] for c in range(3)] + [cref[f"cwT{s}_g"]]
               for s in range(4)]
        ewT = [[cref[f"ewT{s}_{t}"] for t in range(9)] for s in range(4)]
        cb = [sb_from(d[f"cb{s}"], (COMP, 1), f"cb{s}", dt=F32) for s in range(4)]
        eb = [sb_from(d[f"eb{s}"], (96 if s == 2 else 128, 1), f"eb{s}", dt=F32)
              for s in range(4)]

        ident_bf = persist.tile([128, 128], BF16, name="identb", tag="identb")
        nc.vector.tensor_copy(ident_bf[:], cref["ident"][:, :])
        zero_slot = persist.tile([128, DIM], F32R, name="zslot", tag="zslot")
        nc.gpsimd.memset(zero_slot[:].bitcast(F32), 0.0)
        zero_slot_bf = persist.tile([128, DIM], BF16, name="zslotb", tag="zslotb")
        nc.gpsimd.memset(zero_slot_bf[:], 0.0)

        # x_conv[s]: stage-s input, channel-partition layout, 3 chunks of 128.
        # Stages {0,2} and {1,3} have disjoint lifetimes and share buffers
        # via tag reuse (bufs=1 rotation inserts the WAR deps).
        xcpool = top.enter_context(tc.tile_pool(name="xcp", bufs=1))
        # each x_conv level is split at XSPLIT input rows so consumers can
        # start on the top half while the producing stage drains the bottom
        XSPLIT = {0: 16, 1: 32}
        XCOLS = {0: (16 * W_IN[2], (ROWS_IN[2] - 16) * W_IN[2]),
                 1: (32 * W_IN[3], (ROWS_IN[3] - 32) * W_IN[3])}
        x_conv = {}
        for s in range(min(n_stages + 1, 4)):
            x_conv[s] = [[xcpool.tile([128, XCOLS[s % 2][h]], F32R,
                                      name=f"xc{s}_{c}_{h}", tag=f"xc{s % 2}_{c}_{h}")
                          for h in range(2)] for c in range(3)]
        for c in range(3):
            nc.sync.dma_start(x_conv[0][c][0][:, 0:ROWS_IN[0] * W_IN[0]],
                              d["src_conv"][128 * c:128 * (c + 1), :].bitcast(F32R))

        xs1 = [persist.tile([128, DIM], F32R, name=f"xs1_{i}", tag=f"xs1_{i}") for i in range(2)]
        for i in range(2):
            nc.sync.dma_start(xs1[i][:], d["xs1"][:, i, :].bitcast(F32R))
        nc.sync.dma_start(cblob_t[:, csplit:],
                          d["cblob"][:, csplit:].bitcast(F32R))

        # ---- guidance resizes ---------------------------------------------
        g_s = []
        with tc.tile_pool(name="gr", bufs=2) as gr, \
             tc.tile_pool(name="grp", bufs=2, space="PSUM") as grp:
            rref = {}
            rc0 = 0
            for nm, p, c in RSPEC:
                rref[nm] = _Ref(rblob_t, p, rc0, c)
                rc0 += c
            guidT = [[rref[f"gT{c}_{k}"] for k in range(2)] for c in range(3)]
            Cm = [[rref[f"Cm{s}_{k}"] for k in range(2)] for s in range(4)]
            RmT = [[rref[f"RmT{s}_{k}"] for k in range(2)] for s in range(4)]
            for s in range(n_stages):
                W, RI = W_IN[s], ROWS_IN[s]
                for c in range(3):
                    q_ps = grp.tile([W, 224], F32, name="q", tag="q")
                    for k in range(2):
                        nc.tensor.matmul(q_ps[:], Cm[s][k][:],
                                         guidT[c][k][:],
                                         start=(k == 0), stop=(k == 1))
                    q_sb = gr.tile([W, 224], F32R, name="qsb", tag="qsb")
                    nc.scalar.copy(q_sb[:], q_ps[:])
                    qt_sb = [gr.tile([128 if k == 0 else 96, W], F32R, name=f"qt{k}", tag=f"qt{k}")
                             for k in range(2)]
                    for k in range(2):
                        f = 128 if k == 0 else 96
                        t_ps = grp.tile([f, W], F32R, name=f"tp{k}", tag=f"tp{k}")
                        nc.tensor.transpose(t_ps[:], q_sb[:, 128 * k:128 * k + f],
                                            rref["rident"][0:W, 0:W])
                        nc.vector.tensor_copy(qt_sb[k][:], t_ps[:])
                    g_ps = grp.tile([RI, W], F32, name="gps", tag="gps")
                    for k in range(2):
                        nc.tensor.matmul(g_ps[:], RmT[s][k][:],
                                         qt_sb[k][:],
                                         start=(k == 0), stop=(k == 1))
                    g_pix = gr.tile([RI, W], F32, name="gpix", tag="gpix")
                    nc.scalar.copy(g_pix[:], g_ps[:])
                    nc.sync.dma_start(
                        out=bass.AP(tensor=g_dram[s].tensor, offset=c * RI * W,
                                    ap=[[W, RI], [1, W]]),
                        in_=g_pix[:])
                g_s.append(g_dram[s])

        # ---- stages --------------------------------------------------------
        for s in range(n_stages):
            _stage(nc, tc, d, s, x_conv, g_s[s], cwT[s], ewT[s], cb[s], eb[s],
                   sel96, bc96, sel36, bc36, ident, ident_bf, zero_slot,
                   zero_slot_bf, xs1, projT, projTb4, d["guid_hr"], out, n_stages)

        if n_stages < 4:
            ocols = OUT_ROWS[n_stages - 1] * 2 * W_IN[n_stages - 1]
            scol = min(XSPLIT[n_stages % 2] * W_IN[n_stages], ocols)
            for c in range(3):
                nc.sync.dma_start(
                    bass.AP(tensor=out.tensor, offset=c * 128 * ocols,
                            ap=[[ocols, 128], [1, scol]]),
                    x_conv[n_stages][c][0][:, 0:scol].bitcast(F32))
                if ocols > scol:
                    nc.sync.dma_start(
                        bass.AP(tensor=out.tensor, offset=c * 128 * ocols + scol,
                                ap=[[ocols, 128], [1, ocols - scol]]),
                        x_conv[n_stages][c][1][:, 0:ocols - scol].bitcast(F32))

    nc.compile()
    return nc


def _stage(nc, tc, d, s, x_conv, g_s, cwT, ewT, cb, eb, sel96, bc96,
           sel36, bc36, ident, ident_bf, zero_slot, zero_slot_bf, xs1,
           projT, projTb4, guid_hr_dram, out, n_stages):
    W, RI, MR, P, Wblk = W_IN[s], ROWS_IN[s], MROWS[s], PP[s], WBLK[s]
    K = P * Wblk
    WP = W + 2
    n_g = (MR + P - 1) // P
    n_slots = (RI + P - 1) // P
    PW = P * W
    W2 = 2 * W
    xin = x_conv[s]
    XS_IN = {0: 16, 1: 32}[s % 2] * W   # input split column
    final = (s == 3)

    def xin_cols(cc, c0, c1):
        if c1 <= XS_IN:
            return xin[cc][0][:, c0:c1]
        return xin[cc][1][:, c0 - XS_IN:c1 - XS_IN]
    gpg = max(1, 448 // PW)          # groups per mask chunk
    n_chunks = (n_g + gpg - 1) // gpg
    img = d[f"imgz{s}"]
    IMGR = IMG_ROWS[s]
    pq_major = (s != 2)
    sel = sel36 if pq_major else sel96
    bc = bc36 if pq_major else bc96
    nch = 128 if pq_major else 96

    with ExitStack() as ctx:
        stg = ctx.enter_context(tc.tile_pool(name=f"stg{s}", bufs=1))

        comp = stg.tile([COMP, (RI + 2) * WP], F32R, name="comp", tag="comp")
        cpitch = comp.tensor.shape[-1]
        # zero only the conv halo borders; compress overwrites the interior
        nc.gpsimd.memset(comp[:, 0:WP].bitcast(F32), 0.0)
        nc.gpsimd.memset(comp[:, (RI + 1) * WP:(RI + 2) * WP].bitcast(F32), 0.0)
        for bcol in (0, W + 1):
            nc.gpsimd.memset(
                bass.AP(tensor=comp.tensor, offset=comp.offset + WP + bcol,
                        ap=[[cpitch, COMP], [WP, RI], [1, 1]]).bitcast(F32), 0.0)

        # ---- compress conv (own PSUM pool, closed after) -------------------
        with tc.tile_pool(name=f"cps{s}", bufs=2, space="PSUM") as cpool, \
             tc.tile_pool(name=f"gld{s}", bufs=1) as gld:
            gt = gld.tile([3, RI * W], F32R, name="gt", tag="gt")
            nc.scalar.dma_start(gt[:], g_s.bitcast(F32R))
            rows_per = 8 if s == 2 else max(1, 512 // W)
            r0 = 0
            while r0 < RI:
                rn = min(rows_per, RI - r0)
                cps = cpool.tile([COMP, rn * W], F32, name="cps", tag="cps")
                for c in range(3):
                    nc.tensor.matmul(cps[:], cwT[c][:],
                                     xin_cols(c, r0 * W, (r0 + rn) * W),
                                     start=(c == 0), stop=False)
                nc.tensor.matmul(cps[:], cwT[3][:],
                                 gt[:, r0 * W:(r0 + rn) * W], start=False, stop=True)
                dst = bass.AP(tensor=comp.tensor,
                              offset=comp.offset + (r0 + 1) * WP + 1,
                              ap=[[cpitch, COMP], [WP, rn], [1, W]])
                nc.scalar.activation(dst, cps[:].rearrange("p (r w) -> p r w", r=rn),
                                     mybir.ActivationFunctionType.Identity,
                                     bias=cb[:, 0:1], scale=1.0)
                r0 += rn

        # ---- group loop with fused mask pipeline ---------------------------
        band_pool = ctx.enter_context(tc.tile_pool(name=f"bd{s}", bufs=3))
        msk_pool = ctx.enter_context(tc.tile_pool(name=f"mk{s}", bufs=2))
        drn_pool = ctx.enter_context(tc.tile_pool(name=f"drn{s}", bufs=2))
        eps_pool = ctx.enter_context(tc.tile_pool(name=f"eps{s}", bufs=1, space="PSUM"))
        zps_pool = ctx.enter_context(tc.tile_pool(name=f"zps{s}", bufs=1, space="PSUM"))
        rbt_pool = ctx.enter_context(tc.tile_pool(name=f"rbt{s}", bufs=1, space="PSUM"))
        bps_pool = ctx.enter_context(tc.tile_pool(name=f"bps{s}", bufs=1, space="PSUM"))
        tps_pool = ctx.enter_context(tc.tile_pool(name=f"tps{s}", bufs=1 if final else 2, space="PSUM"))
        if pq_major:
            mk2_pool = ctx.enter_context(tc.tile_pool(name=f"mk2_{s}", bufs=1))
        if final:
            ysp_pool = ctx.enter_context(tc.tile_pool(name="ysp", bufs=1, space="PSUM"))
            ysw_pool = ctx.enter_context(tc.tile_pool(name="ysw", bufs=2))
            # per-group guidance+ones rows for the fused bias/guidance matmul
            ght_bufs = []
            for i in range(2):
                gb = stg.tile([4, NB], F32R, name=f"ghtb{i}", tag=f"ghtb{i}")
                nc.scalar.dma_start(gb[3:4, :], d["ones1"].bitcast(F32R))
                ght_bufs.append(gb)

        # persistent slot buffers, zero-padded once
        n_sbuf = min(4, n_slots)
        sdt = BF16 if s >= 2 else F32R
        if s == 0:
            slot_bufs = xs1
            n_sbuf = 2
        else:
            slot_bufs = []
            for i in range(n_sbuf):
                t = stg.tile([128, DIM], sdt, name=f"slotb{i}", tag=f"slotb{i}")
                if s >= 2:
                    nc.gpsimd.memset(t[:], 0.0)
                else:
                    nc.gpsimd.memset(t[:].bitcast(F32), 0.0)
                slot_bufs.append(t)

        slots = {}
        ys_win = {}

        def get_ys_window(w0):
            # 4-row window of proj(x) for stage 3: ys[m][mc, (r-w0)*W + x]
            if w0 in ys_win:
                return ys_win[w0]
            nrows = min(4, RI - w0)
            ncols = nrows * W
            tiles = []
            for m in range(3):
                ps = ysp_pool.tile([128, 448], F32, name="ysps", tag="ysps")
                for cc in range(3):
                    nc.tensor.matmul(ps[:, 0:ncols], projT[cc][:, 128 * m:128 * (m + 1)],
                                     xin_cols(cc, w0 * W, w0 * W + ncols),
                                     start=(cc == 0), stop=(cc == 2))
                t = ysw_pool.tile([128, 448], BF16, name=f"ysw{m}", tag=f"ysw{m}")
                if m % 2 == 0:
                    nc.scalar.copy(t[:, 0:ncols], ps[:, 0:ncols])
                else:
                    nc.vector.tensor_copy(t[:, 0:ncols], ps[:, 0:ncols])
                tiles.append(t)
            ys_win[w0] = tiles
            return tiles

        def get_slot(g):
            if g < 0 or g >= n_slots:
                return zero_slot_bf if s >= 2 else zero_slot
            if g in slots:
                return slots[g]
            t = slot_bufs[g % n_sbuf]
            if s == 0:
                slots[g] = t
                return t
            if final:
                w0 = (g // 4) * 4
                ys = get_ys_window(w0)
                off = (g - w0) * W
                for m in range(3):
                    tp = tps_pool.tile([W, 128], BF16, name="tp", tag="tp")
                    nc.tensor.transpose(tp[:], ys[m][:, off:off + W], ident_bf[:, :])
                    dst = t[0:W, 128 * m:128 * (m + 1)]
                    if (g + m) % 2 == 0:
                        nc.vector.tensor_copy(dst, tp[:])
                    else:
                        nc.scalar.copy(dst, tp[:])
            else:
                for r in range(P):
                    row = g * P + r
                    if row >= RI:
                        break
                    for cc in range(3):
                        tp = tps_pool.tile([W, 128], F32R, name="tp", tag="tp")
                        nc.tensor.transpose(tp[:], xin_cols(cc, row * W, (row + 1) * W),
                                            ident[:, :])
                        dst = t[r * Wblk:r * Wblk + W, 128 * cc:128 * (cc + 1)]
                        if (r + cc) % 2 == 0:
                            nc.vector.tensor_copy(dst, tp[:])
                        else:
                            nc.scalar.copy(dst, tp[:])
            slots[g] = t
            return t

        # mask chunks: chunk ci covers mask rows [ci*gpg*P, ...) -------------
        mchunks = {}

        def get_mchunk(ci):
            if ci in mchunks:
                return mchunks[ci]
            y0 = ci * gpg * P
            yn = min(gpg * P, MR - y0)
            npx = yn * W
            eps_ = eps_pool.tile([nch, gpg * PW], F32, name="eps", tag="eps")
            t = 0
            for di in range(3):
                for dj in range(3):
                    mov = bass.AP(tensor=comp.tensor,
                                  offset=comp.offset + (y0 + di) * WP + dj,
                                  ap=[[cpitch, COMP], [WP, yn], [1, W]])
                    nc.tensor.matmul(eps_[:, 0:npx], ewT[t][:],
                                     mov,
                                     start=(t == 0), stop=(t == 8))
                    t += 1
            mt = msk_pool.tile([nch, gpg * PW], F32R, name="mch", tag="mch")
            nc.scalar.activation(mt[:, 0:npx], eps_[:, 0:npx],
                                 mybir.ActivationFunctionType.Exp,
                                 bias=eb[:, 0:1], scale=1.0)
            zps = zps_pool.tile([4, gpg * PW], F32, name="zps", tag="zps")
            nc.tensor.matmul(zps[:, 0:npx], sel[:],
                             mt[:, 0:npx], start=True, stop=True)
            rzt = drn_pool.tile([4, gpg * PW], F32R, name="rzt", tag="rzt")
            with nc.allow_low_precision(reason="f32r recip; f32-equal bits"):
                nc.vector.reciprocal(rzt[:, 0:npx], zps[:, 0:npx])
            rbt = rbt_pool.tile([nch, gpg * PW], F32, name="rbt", tag="rbt")
            nc.tensor.matmul(rbt[:, 0:npx], bc[:], rzt[:, 0:npx],
                             start=True, stop=True)
            if pq_major:
                # pq-major: mt2[dj*3+di, ((y,x)*4)+pq] = normalized mask
                mt2 = mk2_pool.tile([9, gpg * PW * 4], BF16 if final else F32R,
                                    name="mch2", tag="mch2")
                mp2 = mt2.tensor.shape[-1]
                with nc.allow_low_precision(reason="f32r*f32; f32-equal bits"):
                    for pq in range(4):
                        dst = bass.AP(tensor=mt2.tensor, offset=mt2.offset + pq,
                                      ap=[[mp2, 9], [4, npx]])
                        nc.vector.tensor_mul(dst, mt[32 * pq:32 * pq + 9, 0:npx],
                                             rbt[32 * pq:32 * pq + 9, 0:npx])
                mchunks[ci] = mt2
                return mt2
            mtb = msk_pool.tile([nch, gpg * PW], BF16, name="mchb", tag="mchb")
            with nc.allow_low_precision(reason="bf16 normalized masks"):
                nc.vector.tensor_mul(mtb[:, 0:npx], mt[:, 0:npx], rbt[:, 0:npx])
            mchunks[ci] = mtb
            return mtb

        for g in range(n_g):
            nd = min(P, MR - g * P)
            ci, gl = divmod(g, gpg)
            mt = get_mchunk(ci)
            par_base = (g % 4) * IMGR * NB
            mpitch = mt.tensor.shape[-1]
            # scatter normalized masks into the contiguous band image (DRAM)
            if final:
                for dj, eng in ((0, nc.sync), (1, nc.scalar), (2, nc.sync)):
                    dst = bass.AP(
                        tensor=img.tensor,
                        offset=par_base + dj * NB,
                        ap=[[Wblk * NB, 3], [NB + 4, W], [1, 4]])
                    src = bass.AP(
                        tensor=mt.tensor,
                        offset=mt.offset + (dj * 3) * mpitch + gl * W * 4,
                        ap=[[mpitch, 3], [1, 4 * W]])
                    eng.dma_start(out=dst, in_=src)
            elif pq_major:
                for di in range(3):
                    for dj in range(3):
                        eng = nc.gpsimd if dj == 2 else nc.sync
                        dst = bass.AP(
                            tensor=img.tensor,
                            offset=par_base + (K + (di - 1) * Wblk + dj) * NB,
                            ap=[[Wblk * NB + 4 * W, nd], [NB + 4, W], [1, 4]])
                        p_ = dj * 3 + di
                        src = mt[p_:p_ + 1,
                                 gl * PW * 4:gl * PW * 4 + nd * 4 * W]
                        eng.dma_start(out=dst, in_=src)
            else:
                for di in range(3):
                    for dl in range(nd):
                        X = dl + di - 1
                        eng = nc.gpsimd if (di == 2 and dl == 0) else nc.sync
                        dst = bass.AP(
                            tensor=img.tensor,
                            offset=par_base + (K + X * Wblk) * NB + dl * W,
                            ap=[[PW, 12], [NB + 1, W], [1, 1]])
                        src = bass.AP(
                            tensor=mt.tensor,
                            offset=mt.offset + 32 * di * mpitch + (gl * P + dl) * W,
                            ap=[[mpitch, 12], [1, W], [1, 1]])
                        eng.dma_start(out=dst, in_=src)
            # combined band load: [3K, NB] image rows 1.. -> [K, 3*NB] tile
            bt = band_pool.tile([K, 3 * NB], BF16 if s >= 2 else F32R,
                                name="band", tag="band")
            btp = bt.tensor.shape[-1]
            nc.gpsimd.dma_start(
                out=bass.AP(tensor=bt.tensor, offset=bt.offset,
                            ap=[[btp, K], [NB, 3], [1, NB]]),
                in_=bass.AP(tensor=img.tensor, offset=par_base + NB,
                            ap=[[NB, K], [K * NB, 3], [1, NB]]))
            bps = [bps_pool.tile([128, NB], F32, name=f"bp{cc}", tag=f"bp{cc}")
                   for cc in range(3)]
            if final:
                ght = ght_bufs[g % 2]
                nc.scalar.dma_start(
                    ght[0:3, :], guid_hr_dram[:, 2 * g * WR:(2 * g + 2) * WR].bitcast(F32R))
            for j in range(3):
                xs_t = get_slot(g + j - 1)
                for cc in range(3):
                    nc.tensor.matmul(bps[cc][:],
                                     xs_t[0:K, 128 * cc:128 * (cc + 1)],
                                     bt[0:K, j * NB:(j + 1) * NB],
                                     start=(j == 0), stop=(not final and j == 2))
            if final:
                ghtp = ght.tensor.shape[-1]
                mov = bass.AP(tensor=ght.tensor, offset=ght.offset,
                              ap=[[ghtp, 4], [2, W], [224, 2], [1, 2]])
                for m in range(3):
                    nc.tensor.matmul(bps[m][:], projTb4[:, 128 * m:128 * (m + 1)],
                                     mov, start=False, stop=True)
            # drain
            if not final:
                keep = min(2 * P * (g + 1), ROWS_IN[s + 1]) - 2 * P * g
                osplit = {0: 16, 1: 32}[(s + 1) % 2]
                for cc in range(3):
                    h = 0 if 2 * P * g < osplit else 1
                    tgt = x_conv[s + 1][cc][h]
                    row0 = 2 * P * g - h * osplit
                    dpitch = tgt.tensor.shape[-1]
                    for p in range(2):
                        nrows = (keep - p + 1) // 2
                        if nrows <= 0:
                            continue
                        dst = bass.AP(tensor=tgt.tensor,
                                      offset=tgt.offset + row0 * W2 + p * W2,
                                      ap=[[dpitch, 128], [2 * W2, nrows],
                                          [2, W], [1, 2]])
                        if pq_major:
                            src = bass.AP(tensor=bps[cc].tensor,
                                          offset=bps[cc].offset + p * 2,
                                          ap=[[bps[cc].tensor.shape[-1], 128],
                                              [4 * W, nrows], [4, W], [1, 2]])
                        else:
                            src = bass.AP(tensor=bps[cc].tensor,
                                          offset=bps[cc].offset + p * 2 * PW,
                                          ap=[[bps[cc].tensor.shape[-1], 128],
                                              [W, nrows], [1, W], [PW, 2]])
                        if (cc + p) % 2 == 0:
                            nc.vector.tensor_copy(dst, src)
                        else:
                            nc.scalar.copy(dst, src)
            else:
                # de-interleave (x,p,q) -> rows on 3 engines, store both rows
                osb = drn_pool.tile([128, 3 * NB], BF16, name="osb", tag="osb")
                osp = osb.tensor.shape[-1]
                for m, cp in ((0, nc.scalar.copy), (1, nc.vector.tensor_copy),
                              (2, nc.vector.tensor_copy)):
                    src = bass.AP(tensor=bps[m].tensor, offset=bps[m].offset,
                                  ap=[[bps[m].tensor.shape[-1], 128],
                                      [2, 2], [4, W], [1, 2]])
                    dst = bass.AP(tensor=osb.tensor, offset=osb.offset + m * NB,
                                  ap=[[osp, 128], [224, 2], [2, W], [1, 2]])
                    cp(dst, src)
                nc.gpsimd.dma_start(
                    out=bass.AP(tensor=out.tensor, offset=2 * g * WR,
                                ap=[[HR * WR, 128], [128 * HR * WR, 3], [1, NB]]),
                    in_=bass.AP(tensor=osb.tensor, offset=osb.offset,
                                ap=[[osp, 128], [NB, 3], [1, NB]]))


# ----------------------------------------------------------------------------
# host side
# ----------------------------------------------------------------------------

_NC_CACHE = {}


def _get_nc(n_stages=4):
    if n_stages not in _NC_CACHE:
        _NC_CACHE[n_stages] = build_bass(n_stages)
    return _NC_CACHE[n_stages]


def _pack_xs1(src_half):
    xs = np.zeros((128, 2, DIM), np.float32)
    for slot in range(2):
        for r in range(8):
            row = slot * 8 + r
            if row >= ROWS_IN[0]:
                break
            xs[r * WBLK[0]:r * WBLK[0] + W_IN[0], slot, :] = src_half[:, row, :].T
    return xs


def _core_inputs(source, guidance, weights, core):
    b, half = core // 2, core % 2
    flip = (half == 1)
    src = source[b]
    gd = guidance[b]
    if flip:
        src = src[:, ::-1, :]
        gd = gd[:, ::-1, :]
    src_half = np.ascontiguousarray(src[:, 0:ROWS_IN[0], :]).astype(np.float32)
    perm = _perm_for_core(flip)

    ins = {
        "src_conv": src_half.reshape(DIM, -1),
        "xs1": _pack_xs1(src_half),
        "guid_hr": np.ascontiguousarray(gd[:, 0:HR, :]).reshape(3, -1).astype(np.float32),
        "ones1": np.ones((1, NB), np.float32),
    }
    parts = {"ident": np.eye(128, dtype=np.float32)}
    sel96 = np.zeros((96, 4), np.float32)
    bc96 = np.zeros((4, 96), np.float32)
    for di in range(3):
        for dj in range(3):
            for pq in range(4):
                sel96[di * 32 + dj * 4 + pq, pq] = 1.0
                bc96[pq, di * 32 + dj * 4 + pq] = 1.0
    sel36 = np.zeros((128, 4), np.float32)
    bc36 = np.zeros((4, 128), np.float32)
    for pq in range(4):
        for dj in range(3):
            for di in range(3):
                sel36[pq * 32 + dj * 3 + di, pq] = 1.0
                bc36[pq, pq * 32 + dj * 3 + di] = 1.0
    parts["sel96"], parts["bc96"] = sel96, bc96
    parts["sel36"], parts["bc36"] = sel36, bc36
    pjT = np.asarray(weights["proj_w"])[:, :, 0, 0].T.astype(np.float32)  # [387, 384]
    for c in range(3):
        parts[f"pjT{c}"] = pjT[128 * c:128 * (c + 1)]
    parts["pjTb4"] = np.concatenate(
        [pjT[384:387], np.asarray(weights["proj_b"]).reshape(1, DIM)], axis=0
    ).astype(np.float32)
    rparts = {"rident": np.eye(128, dtype=np.float32)}
    for c in range(3):
        gT = np.ascontiguousarray(gd[c].T).astype(np.float32)
        rparts[f"gT{c}_0"] = gT[0:128]
        rparts[f"gT{c}_1"] = gT[128:224]
    for s in range(4):
        CmT = np.ascontiguousarray(resize_matrix(224, W_IN[s]).T)
        RmTT = np.ascontiguousarray(resize_matrix(224, W_IN[s])[0:ROWS_IN[s]].T)
        rparts[f"Cm{s}_0"], rparts[f"Cm{s}_1"] = CmT[0:128], CmT[128:224]
        rparts[f"RmT{s}_0"], rparts[f"RmT{s}_1"] = RmTT[0:128], RmTT[128:224]
    for s in range(4):
        name = f"up{s + 1}"
        cw = np.asarray(weights[name + "_cw"])[:, :, 0, 0]
        ew = np.asarray(weights[name + "_ew"])
        cwT = np.ascontiguousarray(cw.T).astype(np.float32)   # [387, 64]
        for c in range(3):
            parts[f"cwT{s}_{c}"] = cwT[128 * c:128 * (c + 1)]
        parts[f"cwT{s}_g"] = cwT[384:387]
        ewp = ew[perm]
        if flip:
            ewp = ewp[:, :, ::-1, :]
        ebp = np.asarray(weights[name + "_eb"])[perm]
        if s != 2:
            taps = np.zeros((9, COMP, 128), np.float32)
            eb_ = np.zeros((128, 1), np.float32)
            for t_di in range(3):
                for t_dj in range(3):
                    tap = ewp[:, :, t_di, t_dj].T   # [64, 36] ch' order
                    for di in range(3):
                        for dj in range(3):
                            for pq in range(4):
                                taps[t_di * 3 + t_dj][:, pq * 32 + dj * 3 + di] = \
                                    tap[:, di * 12 + dj * 4 + pq]
            for di in range(3):
                for dj in range(3):
                    for pq in range(4):
                        eb_[pq * 32 + dj * 3 + di, 0] = ebp[di * 12 + dj * 4 + pq]
        else:
            taps = np.zeros((9, COMP, 96), np.float32)
            eb_ = np.zeros((96, 1), np.float32)
            for t_di in range(3):
                for t_dj in range(3):
                    tap = ewp[:, :, t_di, t_dj].T   # [64, 36] ch' order
                    for chp in range(36):
                        di_, rem = divmod(chp, 12)
                        taps[t_di * 3 + t_dj][:, di_ * 32 + rem] = tap[:, chp]
            for di in range(3):
                eb_[32 * di:32 * di + 12, 0] = ebp[12 * di:12 * di + 12]
        for t in range(9):
            parts[f"ewT{s}_{t}"] = taps[t]
        ins[f"eb{s}"] = eb_
        ins[f"cb{s}"] = np.asarray(weights[name + "_cb"]).reshape(COMP, 1).astype(np.float32)
        if s >= 2:
            import ml_dtypes
            ins[f"imgz{s}"] = np.zeros((4, IMG_ROWS[s], NB), ml_dtypes.bfloat16)
        else:
            ins[f"imgz{s}"] = np.zeros((4, IMG_ROWS[s], NB), np.float32)

    def pack(spec, pieces):
        cols = sum(c for _, _, c in spec)
        blob = np.zeros((128, cols), np.float32)
        c0 = 0
        for nm, p, c in spec:
            a = pieces[nm]
            assert a.shape == (p, c), (nm, a.shape, (p, c))
            blob[0:p, c0:c0 + c] = a
            c0 += c
        return blob

    ins["cblob"] = pack(_cblob_spec(), parts)
    ins["rblob"] = pack(_rblob_spec(), rparts)
    return ins


def kernel(source, guidance, _trace=False, **w):
    source = np.asarray(source, dtype=np.float32)
    guidance = np.asarray(guidance, dtype=np.float32)
    nc = _get_nc(4)
    in_maps = [_core_inputs(source, guidance, w, core) for core in range(8)]
    try:
        res = run_bass_kernel_spmd(nc, in_maps, core_ids=list(range(8)), trace=_trace)
    except ModuleNotFoundError:
        res = run_bass_kernel_spmd(nc, in_maps, core_ids=list(range(8)))
    if _trace and res.exec_time_ns is not None:
        print(f"HW exec time: {res.exec_time_ns} ns", flush=True)
        if res.instructions_and_trace:
            print("trace:", res.instructions_and_trace[1], flush=True)
    full = np.zeros((4, DIM, 224, 224), np.float32)
    for core in range(8):
        b, half = core // 2, core % 2
        o = np.asarray(res.results[core]["out"], dtype=np.float32)
        if half == 0:
            full[b, :, 0:HR, :] = o
        else:
            full[b, :, HR:224, :] = o[:, ::-1, :]
    return full


# revision 59
# speedup vs baseline: 1.0269x; 1.0269x over previous
"""CarafeGuidedUpsampler Trainium2 kernel (v2).

Sharding: 8 cores = 4 samples x 2 vertical halves. Bottom halves are
row-flipped on the host (with a matching encoder-weight permutation) so all
cores run one SPMD program: "compute output rows [0,112) from source rows
[0,9) + guidance".

Per-core device program (fp32 data, matmuls as float32r):
  4x CARAFE stage: compress 1x1 conv (PE) -> encoder 3x3 conv + exp (PE+ACT)
  -> softmax normalize (PE k-sum, DVE recip, PE broadcast matmul, DVE mul) ->
  CARAFE reassembly as banded matmuls: normalized masks are scattered into a
  contiguous 3-slot band image in DRAM (diagonal access patterns are only
  legal on the DRAM side), loaded back as one [K, 3*NB] tile, and contracted
  against column-partition "xS" slots with PSUM accumulation over the 3
  vertical taps.

Stage 4 fuses the final 1x1 projection through the reassembly: slots hold
P^T-projected pixels (ys = proj(x)), so the band matmuls emit projected
output channels directly; the 3 guidance channels + bias enter as a K=4
matmul against per-group guidance rows (with a constant ones row). Stages
0/1/3 use a pq-major mask-channel layout (ch = pq*32 + dj*3 + di) so the
scatter writes 4-element contiguous runs (4x fewer DMA descriptors); the
stage-2/3 band path, slots and the output store are bf16 to halve DMA
transfer time (output converted back to f32 on the host).

v2 performance notes (TimelineSim cost model, per core):
  745675 ns (v1) -> 434714 ns. Main levers: DMA count 979 -> ~520 (HWDGE
  and DMA_ENGINES are exclusive devices; ~630ns HWDGE per DMA), band loads
  and output stores moved to the Pool SWDGE queue (bypasses HWDGE),
  PE-broadcast softmax normalization instead of a DRAM round trip,
  constant blobs (2 DMAs instead of ~58), proj fusion (-94us PE),
  zero-once slot buffers, halo-only comp memsets, and parity-4 band
  images with 3-deep band buffering for scatter->load pipelining.
"""

import numpy as np
from contextlib import ExitStack

import concourse.bass as bass
import concourse.tile as tile
from concourse import bacc, mybir
from concourse.bass_utils import run_bass_kernel_spmd

F32 = mybir.dt.float32
F32R = mybir.dt.float32r
BF16 = mybir.dt.bfloat16

DIM = 384
COMP = 64
CIN = 387

# per-stage geometry (stage index 0..3)
W_IN = [14, 28, 56, 112]          # input width
ROWS_IN = [9, 16, 30, 57]         # input rows incl. halo
MROWS = [8, 15, 29, 56]           # mask rows computed
PP = [8, 4, 2, 1]                 # input rows packed per xS slot
WBLK = [16, 32, 64, 113]          # partition stride per packed row
NB = 448                          # band matmul N = 4*P*W (equal all stages)
KK_ = [PP[s] * WBLK[s] for s in range(4)]   # [128, 128, 128, 113]
OUT_ROWS = [16, 30, 57, 112]      # output rows kept (== ROWS_IN[s+1])
HR, WR = 112, 224
IMG_ROWS = [1 + 3 * KK_[s] for s in range(4)]


def resize_matrix(n_in, n_out):
    """Row-resize matrix matching jax.image.resize(method='bilinear')."""
    scale = n_out / n_in
    j = np.arange(n_in, dtype=np.float64)
    out = np.zeros((n_out, n_in), np.float64)
    for i in range(n_out):
        center = (i + 0.5) / scale - 0.5
        w = np.maximum(0.0, 1.0 - np.abs((j - center) * scale))
        out[i] = w / w.sum()
    return out.astype(np.float32)


def _perm_for_core(flip):
    """perm[ch'] = original encoder out-channel; ch' = di*12+dj*4+p*2+q."""
    perm = np.zeros(36, np.int64)
    for di in range(3):
        for dj in range(3):
            for p in range(2):
                for q in range(2):
                    sdi, sp = (2 - di, 1 - p) if flip else (di, p)
                    perm[di * 12 + dj * 4 + p * 2 + q] = (sdi * 3 + dj) * 4 + sp * 2 + q
    return perm


def _cblob_spec():
    """(name, partitions, cols) of the packed f32 constant blob, in order.
    Constants needed by stages 0/1 come first so the blob can be loaded in
    two DMAs with the early half gating less work."""
    spec = [("ident", 128, 128), ("sel96", 96, 4), ("bc96", 4, 96),
            ("sel36", 128, 4), ("bc36", 4, 128)]
    for s in range(4):
        for c in range(3):
            spec.append((f"cwT{s}_{c}", 128, COMP))
        spec.append((f"cwT{s}_g", 3, COMP))
        nch = 96 if s == 2 else 128
        for t in range(9):
            spec.append((f"ewT{s}_{t}", COMP, nch))
        if s == 1:
            spec += [("pjT0", 128, DIM), ("pjT1", 128, DIM), ("pjT2", 128, DIM),
                     ("pjTb4", 4, DIM)]
    return spec


def _cblob_split():
    """Column where the early/late cblob halves divide (end of s1 weights)."""
    c0 = 0
    for nm, _, c in _cblob_spec():
        c0 += c
        if nm == "ewT1_8":
            return c0
    raise AssertionError


def _rblob_spec():
    """Constants used only by the guidance-resize phase."""
    spec = [("rident", 128, 128)]
    for c in range(3):
        for k in range(2):
            spec.append((f"gT{c}_{k}", 128 if k == 0 else 96, 224))
    for s in range(4):
        for k in range(2):
            spec.append((f"Cm{s}_{k}", 128 if k == 0 else 96, W_IN[s]))
        for k in range(2):
            spec.append((f"RmT{s}_{k}", 128 if k == 0 else 96, ROWS_IN[s]))
    return spec


class _Ref:
    """Column-offset view into a packed blob tile, sliceable like a tile."""

    def __init__(self, tile_, p, c0, cols):
        self.t, self.p, self.c0, self.cols = tile_, p, c0, cols

    def __getitem__(self, idx):
        if not isinstance(idx, tuple):
            idx = (idx, slice(None))
        ps, cs = idx
        p0 = ps.start if ps.start is not None else 0
        p1 = ps.stop if ps.stop is not None else self.p
        c0 = cs.start if cs.start is not None else 0
        c1 = cs.stop if cs.stop is not None else self.cols
        return self.t[p0:p1, self.c0 + c0:self.c0 + c1]


# ----------------------------------------------------------------------------
# device program
# ----------------------------------------------------------------------------

def build_bass(n_stages=4):
    nc = bacc.Bacc("TRN2", target_bir_lowering=False, debug=False,
                   enable_asserts=False, num_devices=8)
    d = {}

    def din(name, shape, dt=F32):
        d[name] = nc.dram_tensor(name, list(shape), dt, kind="ExternalInput").ap()
        return d[name]

    din("src_conv", (DIM, ROWS_IN[0] * W_IN[0]))
    din("xs1", (128, 2, DIM))
    din("guid_hr", (3, HR * WR))
    din("ones1", (1, NB))
    CSPEC, RSPEC = _cblob_spec(), _rblob_spec()
    din("cblob", (128, sum(c for _, _, c in CSPEC)))
    din("rblob", (128, sum(c for _, _, c in RSPEC)))
    for s in range(4):
        nch = 96 if s == 2 else 128
        din(f"cb{s}", (COMP, 1))
        din(f"eb{s}", (nch, 1))
        din(f"imgz{s}", (4, IMG_ROWS[s], NB), dt=(F32R if s <= 1 else BF16))

    if n_stages == 4:
        out = nc.dram_tensor("out", [DIM, HR, WR], BF16, kind="ExternalOutput").ap()
    else:
        s = n_stages - 1
        out = nc.dram_tensor(
            "out", [DIM, OUT_ROWS[s], 2 * W_IN[s]], F32, kind="ExternalOutput").ap()

    g_dram = [nc.dram_tensor(f"gd{s}", [3, ROWS_IN[s] * W_IN[s]], F32).ap()
              for s in range(4)]

    with tile.TileContext(nc) as tc, ExitStack() as top:
        const = top.enter_context(tc.tile_pool(name="const", bufs=1))
        persist = top.enter_context(tc.tile_pool(name="persist", bufs=1))

        def sb_from(dram_ap, shape, name, dt=F32R):
            t = const.tile(list(shape), dt, name=name, tag=name)
            nc.sync.dma_start(t[:], dram_ap.bitcast(dt) if dt is F32R else dram_ap)
            return t

        grc = top.enter_context(tc.tile_pool(name="grc", bufs=1))
        rblob_t = grc.tile([128, sum(c for _, _, c in RSPEC)], F32R,
                           name="rblob", tag="rblob")
        nc.scalar.dma_start(rblob_t[:], d["rblob"].bitcast(F32R))
        cblob_t = const.tile([128, sum(c for _, _, c in CSPEC)], F32R,
                             name="cblob", tag="cblob")
        csplit = _cblob_split()
        nc.sync.dma_start(cblob_t[:, 0:csplit],
                          d["cblob"][:, 0:csplit].bitcast(F32R))
        cref = {}
        cc0 = 0
        for nm, p, c in CSPEC:
            cref[nm] = _Ref(cblob_t, p, cc0, c)
            cc0 += c
        ident, sel96, bc96 = cref["ident"], cref["sel96"], cref["bc96"]
        sel36, bc36 = cref["sel36"], cref["bc36"]
        projT = [cref[f"pjT{c}"] for c in range(3)]
        projTb4 = cref["pjTb4"]
        cwT = [[cref[f"cwT{s}_{c}"] for c in range(3)] + [cref[f"cwT{s}_g"]]
               for s in range(4)]
        ewT = [[cref[f"ewT{s}_{t}"] for t in range(9)] for s in range(4)]
        cb = [sb_from(d[f"cb{s}"], (COMP, 1), f"cb{s}", dt=F32) for s in range(4)]
        eb = [sb_from(d[f"eb{s}"], (96 if s == 2 else 128, 1), f"eb{s}", dt=F32)
              for s in range(4)]

        ident_bf = persist.tile([128, 128], BF16, name="identb", tag="identb")
        nc.vector.tensor_copy(ident_bf[:], cref["ident"][:, :])
        zero_slot = persist.tile([128, DIM], F32R, name="zslot", tag="zslot")
        nc.gpsimd.memset(zero_slot[:].bitcast(F32), 0.0)
        zero_slot_bf = persist.tile([128, DIM], BF16, name="zslotb", tag="zslotb")
        nc.gpsimd.memset(zero_slot_bf[:], 0.0)

        # x_conv[s]: stage-s input, channel-partition layout, 3 chunks of 128.
        # Stages {0,2} and {1,3} have disjoint lifetimes and share buffers
        # via tag reuse (bufs=1 rotation inserts the WAR deps).
        xcpool = top.enter_context(tc.tile_pool(name="xcp", bufs=1))
        # each x_conv level is split at XSPLIT input rows so consumers can
        # start on the top half while the producing stage drains the bottom
        XSPLIT = {0: 16, 1: 32}
        XCOLS = {0: (16 * W_IN[2], (ROWS_IN[2] - 16) * W_IN[2]),
                 1: (32 * W_IN[3], (ROWS_IN[3] - 32) * W_IN[3])}
        x_conv = {}
        for s in range(min(n_stages + 1, 4)):
            x_conv[s] = [[xcpool.tile([128, XCOLS[s % 2][h]], F32R,
                                      name=f"xc{s}_{c}_{h}", tag=f"xc{s % 2}_{c}_{h}")
                          for h in range(2)] for c in range(3)]
        for c in range(3):
            nc.sync.dma_start(x_conv[0][c][0][:, 0:ROWS_IN[0] * W_IN[0]],
                              d["src_conv"][128 * c:128 * (c + 1), :].bitcast(F32R))

        xs1 = [persist.tile([128, DIM], F32R, name=f"xs1_{i}", tag=f"xs1_{i}") for i in range(2)]
        for i in range(2):
            nc.sync.dma_start(xs1[i][:], d["xs1"][:, i, :].bitcast(F32R))
        nc.sync.dma_start(cblob_t[:, csplit:],
                          d["cblob"][:, csplit:].bitcast(F32R))

        # ---- guidance resizes ---------------------------------------------
        g_s = []
        with tc.tile_pool(name="gr", bufs=2) as gr, \
             tc.tile_pool(name="grp", bufs=2, space="PSUM") as grp:
            rref = {}
            rc0 = 0
            for nm, p, c in RSPEC:
                rref[nm] = _Ref(rblob_t, p, rc0, c)
                rc0 += c
            guidT = [[rref[f"gT{c}_{k}"] for k in range(2)] for c in range(3)]
            Cm = [[rref[f"Cm{s}_{k}"] for k in range(2)] for s in range(4)]
            RmT = [[rref[f"RmT{s}_{k}"] for k in range(2)] for s in range(4)]
            for s in range(n_stages):
                W, RI = W_IN[s], ROWS_IN[s]
                for c in range(3):
                    q_ps = grp.tile([W, 224], F32, name="q", tag="q")
                    for k in range(2):
                        nc.tensor.matmul(q_ps[:], Cm[s][k][:],
                                         guidT[c][k][:],
                                         start=(k == 0), stop=(k == 1))
                    q_sb = gr.tile([W, 224], F32R, name="qsb", tag="qsb")
                    nc.scalar.copy(q_sb[:], q_ps[:])
                    qt_sb = [gr.tile([128 if k == 0 else 96, W], F32R, name=f"qt{k}", tag=f"qt{k}")
                             for k in range(2)]
                    for k in range(2):
                        f = 128 if k == 0 else 96
                        t_ps = grp.tile([f, W], F32R, name=f"tp{k}", tag=f"tp{k}")
                        nc.tensor.transpose(t_ps[:], q_sb[:, 128 * k:128 * k + f],
                                            rref["rident"][0:W, 0:W])
                        nc.vector.tensor_copy(qt_sb[k][:], t_ps[:])
                    g_ps = grp.tile([RI, W], F32, name="gps", tag="gps")
                    for k in range(2):
                        nc.tensor.matmul(g_ps[:], RmT[s][k][:],
                                         qt_sb[k][:],
                                         start=(k == 0), stop=(k == 1))
                    g_pix = gr.tile([RI, W], F32, name="gpix", tag="gpix")
                    nc.scalar.copy(g_pix[:], g_ps[:])
                    nc.sync.dma_start(
                        out=bass.AP(tensor=g_dram[s].tensor, offset=c * RI * W,
                                    ap=[[W, RI], [1, W]]),
                        in_=g_pix[:])
                g_s.append(g_dram[s])

        # ---- stages --------------------------------------------------------
        for s in range(n_stages):
            _stage(nc, tc, d, s, x_conv, g_s[s], cwT[s], ewT[s], cb[s], eb[s],
                   sel96, bc96, sel36, bc36, ident, ident_bf, zero_slot,
                   zero_slot_bf, xs1, projT, projTb4, d["guid_hr"], out, n_stages)

        if n_stages < 4:
            ocols = OUT_ROWS[n_stages - 1] * 2 * W_IN[n_stages - 1]
            scol = min(XSPLIT[n_stages % 2] * W_IN[n_stages], ocols)
            for c in range(3):
                nc.sync.dma_start(
                    bass.AP(tensor=out.tensor, offset=c * 128 * ocols,
                            ap=[[ocols, 128], [1, scol]]),
                    x_conv[n_stages][c][0][:, 0:scol].bitcast(F32))
                if ocols > scol:
                    nc.sync.dma_start(
                        bass.AP(tensor=out.tensor, offset=c * 128 * ocols + scol,
                                ap=[[ocols, 128], [1, ocols - scol]]),
                        x_conv[n_stages][c][1][:, 0:ocols - scol].bitcast(F32))

    nc.compile()
    return nc


def _stage(nc, tc, d, s, x_conv, g_s, cwT, ewT, cb, eb, sel96, bc96,
           sel36, bc36, ident, ident_bf, zero_slot, zero_slot_bf, xs1,
           projT, projTb4, guid_hr_dram, out, n_stages):
    W, RI, MR, P, Wblk = W_IN[s], ROWS_IN[s], MROWS[s], PP[s], WBLK[s]
    K = P * Wblk
    WP = W + 2
    n_g = (MR + P - 1) // P
    n_slots = (RI + P - 1) // P
    PW = P * W
    W2 = 2 * W
    xin = x_conv[s]
    XS_IN = {0: 16, 1: 32}[s % 2] * W   # input split column
    final = (s == 3)

    def xin_cols(cc, c0, c1):
        if c1 <= XS_IN:
            return xin[cc][0][:, c0:c1]
        return xin[cc][1][:, c0 - XS_IN:c1 - XS_IN]
    gpg = max(1, 448 // PW)          # groups per mask chunk
    n_chunks = (n_g + gpg - 1) // gpg
    img = d[f"imgz{s}"]
    IMGR = IMG_ROWS[s]
    pq_major = (s != 2)
    sel = sel36 if pq_major else sel96
    bc = bc36 if pq_major else bc96
    nch = 128 if pq_major else 96

    with ExitStack() as ctx:
        stg = ctx.enter_context(tc.tile_pool(name=f"stg{s}", bufs=1))

        comp = stg.tile([COMP, (RI + 2) * WP], F32R, name="comp", tag="comp")
        cpitch = comp.tensor.shape[-1]
        # zero only the conv halo borders; compress overwrites the interior
        nc.gpsimd.memset(comp[:, 0:WP].bitcast(F32), 0.0)
        nc.gpsimd.memset(comp[:, (RI + 1) * WP:(RI + 2) * WP].bitcast(F32), 0.0)
        for bcol in (0, W + 1):
            nc.gpsimd.memset(
                bass.AP(tensor=comp.tensor, offset=comp.offset + WP + bcol,
                        ap=[[cpitch, COMP], [WP, RI], [1, 1]]).bitcast(F32), 0.0)

        # ---- compress conv (own PSUM pool, closed after) -------------------
        with tc.tile_pool(name=f"cps{s}", bufs=2, space="PSUM") as cpool, \
             tc.tile_pool(name=f"gld{s}", bufs=1) as gld:
            gt = gld.tile([3, RI * W], F32R, name="gt", tag="gt")
            nc.scalar.dma_start(gt[:], g_s.bitcast(F32R))
            rows_per = 8 if s == 2 else max(1, 512 // W)
            r0 = 0
            while r0 < RI:
                rn = min(rows_per, RI - r0)
                cps = cpool.tile([COMP, rn * W], F32, name="cps", tag="cps")
                for c in range(3):
                    nc.tensor.matmul(cps[:], cwT[c][:],
                                     xin_cols(c, r0 * W, (r0 + rn) * W),
                                     start=(c == 0), stop=False)
                nc.tensor.matmul(cps[:], cwT[3][:],
                                 gt[:, r0 * W:(r0 + rn) * W], start=False, stop=True)
                dst = bass.AP(tensor=comp.tensor,
                              offset=comp.offset + (r0 + 1) * WP + 1,
                              ap=[[cpitch, COMP], [WP, rn], [1, W]])
                nc.scalar.activation(dst, cps[:].rearrange("p (r w) -> p r w", r=rn),
                                     mybir.ActivationFunctionType.Identity,
                                     bias=cb[:, 0:1], scale=1.0)
                r0 += rn

        # ---- group loop with fused mask pipeline ---------------------------
        band_pool = ctx.enter_context(tc.tile_pool(name=f"bd{s}", bufs=3))
        msk_pool = ctx.enter_context(tc.tile_pool(name=f"mk{s}", bufs=2))
        drn_pool = ctx.enter_context(tc.tile_pool(name=f"drn{s}", bufs=2))
        eps_pool = ctx.enter_context(tc.tile_pool(name=f"eps{s}", bufs=1, space="PSUM"))
        zps_pool = ctx.enter_context(tc.tile_pool(name=f"zps{s}", bufs=1, space="PSUM"))
        rbt_pool = ctx.enter_context(tc.tile_pool(name=f"rbt{s}", bufs=1, space="PSUM"))
        bps_pool = ctx.enter_context(tc.tile_pool(name=f"bps{s}", bufs=1, space="PSUM"))
        tps_pool = ctx.enter_context(tc.tile_pool(name=f"tps{s}", bufs=1 if final else 2, space="PSUM"))
        if pq_major:
            mk2_pool = ctx.enter_context(tc.tile_pool(name=f"mk2_{s}", bufs=1))
        if final:
            ysp_pool = ctx.enter_context(tc.tile_pool(name="ysp", bufs=1, space="PSUM"))
            ysw_pool = ctx.enter_context(tc.tile_pool(name="ysw", bufs=2))
            # per-group guidance+ones rows for the fused bias/guidance matmul
            ght_bufs = []
            for i in range(2):
                gb = stg.tile([4, NB], F32R, name=f"ghtb{i}", tag=f"ghtb{i}")
                nc.scalar.dma_start(gb[3:4, :], d["ones1"].bitcast(F32R))
                ght_bufs.append(gb)

        # persistent slot buffers, zero-padded once
        n_sbuf = min(4, n_slots)
        sdt = BF16 if s >= 2 else F32R
        if s == 0:
            slot_bufs = xs1
            n_sbuf = 2
        else:
            slot_bufs = []
            for i in range(n_sbuf):
                t = stg.tile([128, DIM], sdt, name=f"slotb{i}", tag=f"slotb{i}")
                if s >= 2:
                    nc.gpsimd.memset(t[:], 0.0)
                else:
                    nc.gpsimd.memset(t[:].bitcast(F32), 0.0)
                slot_bufs.append(t)

        slots = {}
        ys_win = {}

        def get_ys_window(w0):
            # 4-row window of proj(x) for stage 3: ys[m][mc, (r-w0)*W + x]
            if w0 in ys_win:
                return ys_win[w0]
            nrows = min(4, RI - w0)
            ncols = nrows * W
            tiles = []
            for m in range(3):
                ps = ysp_pool.tile([128, 448], F32, name="ysps", tag="ysps")
                for cc in range(3):
                    nc.tensor.matmul(ps[:, 0:ncols], projT[cc][:, 128 * m:128 * (m + 1)],
                                     xin_cols(cc, w0 * W, w0 * W + ncols),
                                     start=(cc == 0), stop=(cc == 2))
                t = ysw_pool.tile([128, 448], BF16, name=f"ysw{m}", tag=f"ysw{m}")
                if m % 2 == 0:
                    nc.scalar.copy(t[:, 0:ncols], ps[:, 0:ncols])
                else:
                    nc.vector.tensor_copy(t[:, 0:ncols], ps[:, 0:ncols])
                tiles.append(t)
            ys_win[w0] = tiles
            return tiles

        def get_slot(g):
            if g < 0 or g >= n_slots:
                return zero_slot_bf if s >= 2 else zero_slot
            if g in slots:
                return slots[g]
            t = slot_bufs[g % n_sbuf]
            if s == 0:
                slots[g] = t
                return t
            if final:
                w0 = (g // 4) * 4
                ys = get_ys_window(w0)
                off = (g - w0) * W
                for m in range(3):
                    tp = tps_pool.tile([W, 128], BF16, name="tp", tag="tp")
                    nc.tensor.transpose(tp[:], ys[m][:, off:off + W], ident_bf[:, :])
                    dst = t[0:W, 128 * m:128 * (m + 1)]
                    if (g + m) % 2 == 0:
                        nc.vector.tensor_copy(dst, tp[:])
                    else:
                        nc.scalar.copy(dst, tp[:])
            else:
                for r in range(P):
                    row = g * P + r
                    if row >= RI:
                        break
                    for cc in range(3):
                        tp = tps_pool.tile([W, 128], F32R, name="tp", tag="tp")
                        nc.tensor.transpose(tp[:], xin_cols(cc, row * W, (row + 1) * W),
                                            ident[:, :])
                        dst = t[r * Wblk:r * Wblk + W, 128 * cc:128 * (cc + 1)]
                        if (r + cc) % 2 == 0:
                            nc.vector.tensor_copy(dst, tp[:])
                        else:
                            nc.scalar.copy(dst, tp[:])
            slots[g] = t
            return t

        # mask chunks: chunk ci covers mask rows [ci*gpg*P, ...) -------------
        mchunks = {}

        def get_mchunk(ci):
            if ci in mchunks:
                return mchunks[ci]
            y0 = ci * gpg * P
            yn = min(gpg * P, MR - y0)
            npx = yn * W
            eps_ = eps_pool.tile([nch, gpg * PW], F32, name="eps", tag="eps")
            t = 0
            for di in range(3):
                for dj in range(3):
                    mov = bass.AP(tensor=comp.tensor,
                                  offset=comp.offset + (y0 + di) * WP + dj,
                                  ap=[[cpitch, COMP], [WP, yn], [1, W]])
                    nc.tensor.matmul(eps_[:, 0:npx], ewT[t][:],
                                     mov,
                                     start=(t == 0), stop=(t == 8))
                    t += 1
            mt = msk_pool.tile([nch, gpg * PW], F32R, name="mch", tag="mch")
            nc.scalar.activation(mt[:, 0:npx], eps_[:, 0:npx],
                                 mybir.ActivationFunctionType.Exp,
                                 bias=eb[:, 0:1], scale=1.0)
            zps = zps_pool.tile([4, gpg * PW], F32, name="zps", tag="zps")
            nc.tensor.matmul(zps[:, 0:npx], sel[:],
                             mt[:, 0:npx], start=True, stop=True)
            rzt = drn_pool.tile([4, gpg * PW], F32R, name="rzt", tag="rzt")
            with nc.allow_low_precision(reason="f32r recip; f32-equal bits"):
                nc.vector.reciprocal(rzt[:, 0:npx], zps[:, 0:npx])
            rbt = rbt_pool.tile([nch, gpg * PW], F32, name="rbt", tag="rbt")
            nc.tensor.matmul(rbt[:, 0:npx], bc[:], rzt[:, 0:npx],
                             start=True, stop=True)
            if pq_major:
                # pq-major: mt2[dj*3+di, ((y,x)*4)+pq] = normalized mask
                mt2 = mk2_pool.tile([9, gpg * PW * 4], BF16 if final else F32R,
                                    name="mch2", tag="mch2")
                mp2 = mt2.tensor.shape[-1]
                with nc.allow_low_precision(reason="f32r*f32; f32-equal bits"):
                    for pq in range(4):
                        dst = bass.AP(tensor=mt2.tensor, offset=mt2.offset + pq,
                                      ap=[[mp2, 9], [4, npx]])
                        nc.vector.tensor_mul(dst, mt[32 * pq:32 * pq + 9, 0:npx],
                                             rbt[32 * pq:32 * pq + 9, 0:npx])
                mchunks[ci] = mt2
                return mt2
            mtb = msk_pool.tile([nch, gpg * PW], BF16, name="mchb", tag="mchb")
            with nc.allow_low_precision(reason="bf16 normalized masks"):
                nc.vector.tensor_mul(mtb[:, 0:npx], mt[:, 0:npx], rbt[:, 0:npx])
            mchunks[ci] = mtb
            return mtb

        for g in range(n_g):
            nd = min(P, MR - g * P)
            ci, gl = divmod(g, gpg)
            mt = get_mchunk(ci)
            par_base = (g % 4) * IMGR * NB
            mpitch = mt.tensor.shape[-1]
            # scatter normalized masks into the contiguous band image (DRAM)
            if final:
                for dj, eng in ((0, nc.sync), (1, nc.sync), (2, nc.sync)):
                    dst = bass.AP(
                        tensor=img.tensor,
                        offset=par_base + dj * NB,
                        ap=[[Wblk * NB, 3], [NB + 4, W], [1, 4]])
                    src = bass.AP(
                        tensor=mt.tensor,
                        offset=mt.offset + (dj * 3) * mpitch + gl * W * 4,
                        ap=[[mpitch, 3], [1, 4 * W]])
                    eng.dma_start(out=dst, in_=src)
            elif pq_major:
                for di in range(3):
                    for dj in range(3):
                        eng = nc.gpsimd if dj == 2 else nc.sync
                        dst = bass.AP(
                            tensor=img.tensor,
                            offset=par_base + (K + (di - 1) * Wblk + dj) * NB,
                            ap=[[Wblk * NB + 4 * W, nd], [NB + 4, W], [1, 4]])
                        p_ = dj * 3 + di
                        src = mt[p_:p_ + 1,
                                 gl * PW * 4:gl * PW * 4 + nd * 4 * W]
                        eng.dma_start(out=dst, in_=src)
            else:
                for di in range(3):
                    for dl in range(nd):
                        X = dl + di - 1
                        eng = nc.gpsimd if (di == 2 and dl == 0) else nc.sync
                        dst = bass.AP(
                            tensor=img.tensor,
                            offset=par_base + (K + X * Wblk) * NB + dl * W,
                            ap=[[PW, 12], [NB + 1, W], [1, 1]])
                        src = bass.AP(
                            tensor=mt.tensor,
                            offset=mt.offset + 32 * di * mpitch + (gl * P + dl) * W,
                            ap=[[mpitch, 12], [1, W], [1, 1]])
                        eng.dma_start(out=dst, in_=src)
            # combined band load: [3K, NB] image rows 1.. -> [K, 3*NB] tile
            bt = band_pool.tile([K, 3 * NB], BF16 if s >= 2 else F32R,
                                name="band", tag="band")
            btp = bt.tensor.shape[-1]
            nc.gpsimd.dma_start(
                out=bass.AP(tensor=bt.tensor, offset=bt.offset,
                            ap=[[btp, K], [NB, 3], [1, NB]]),
                in_=bass.AP(tensor=img.tensor, offset=par_base + NB,
                            ap=[[NB, K], [K * NB, 3], [1, NB]]))
            bps = [bps_pool.tile([128, NB], F32, name=f"bp{cc}", tag=f"bp{cc}")
                   for cc in range(3)]
            if final:
                ght = ght_bufs[g % 2]
                nc.scalar.dma_start(
                    ght[0:3, :], guid_hr_dram[:, 2 * g * WR:(2 * g + 2) * WR].bitcast(F32R))
            for j in range(3):
                xs_t = get_slot(g + j - 1)
                for cc in range(3):
                    nc.tensor.matmul(bps[cc][:],
                                     xs_t[0:K, 128 * cc:128 * (cc + 1)],
                                     bt[0:K, j * NB:(j + 1) * NB],
                                     start=(j == 0), stop=(not final and j == 2))
            if final:
                ghtp = ght.tensor.shape[-1]
                mov = bass.AP(tensor=ght.tensor, offset=ght.offset,
                              ap=[[ghtp, 4], [2, W], [224, 2], [1, 2]])
                for m in range(3):
                    nc.tensor.matmul(bps[m][:], projTb4[:, 128 * m:128 * (m + 1)],
                                     mov, start=False, stop=True)
            # drain
            if not final:
                keep = min(2 * P * (g + 1), ROWS_IN[s + 1]) - 2 * P * g
                osplit = {0: 16, 1: 32}[(s + 1) % 2]
                for cc in range(3):
                    h = 0 if 2 * P * g < osplit else 1
                    tgt = x_conv[s + 1][cc][h]
                    row0 = 2 * P * g - h * osplit
                    dpitch = tgt.tensor.shape[-1]
                    for p in range(2):
                        nrows = (keep - p + 1) // 2
                        if nrows <= 0:
                            continue
                        dst = bass.AP(tensor=tgt.tensor,
                                      offset=tgt.offset + row0 * W2 + p * W2,
                                      ap=[[dpitch, 128], [2 * W2, nrows],
                                          [2, W], [1, 2]])
                        if pq_major:
                            src = bass.AP(tensor=bps[cc].tensor,
                                          offset=bps[cc].offset + p * 2,
                                          ap=[[bps[cc].tensor.shape[-1], 128],
                                              [4 * W, nrows], [4, W], [1, 2]])
                        else:
                            src = bass.AP(tensor=bps[cc].tensor,
                                          offset=bps[cc].offset + p * 2 * PW,
                                          ap=[[bps[cc].tensor.shape[-1], 128],
                                              [W, nrows], [1, W], [PW, 2]])
                        if (cc + p) % 2 == 0:
                            nc.vector.tensor_copy(dst, src)
                        else:
                            nc.scalar.copy(dst, src)
            else:
                # de-interleave (x,p,q) -> rows on 3 engines, store both rows
                osb = drn_pool.tile([128, 3 * NB], BF16, name="osb", tag="osb")
                osp = osb.tensor.shape[-1]
                for m, cp in ((0, nc.scalar.copy), (1, nc.vector.tensor_copy),
                              (2, nc.vector.tensor_copy)):
                    src = bass.AP(tensor=bps[m].tensor, offset=bps[m].offset,
                                  ap=[[bps[m].tensor.shape[-1], 128],
                                      [2, 2], [4, W], [1, 2]])
                    dst = bass.AP(tensor=osb.tensor, offset=osb.offset + m * NB,
                                  ap=[[osp, 128], [224, 2], [2, W], [1, 2]])
                    cp(dst, src)
                nc.gpsimd.dma_start(
                    out=bass.AP(tensor=out.tensor, offset=2 * g * WR,
                                ap=[[HR * WR, 128], [128 * HR * WR, 3], [1, NB]]),
                    in_=bass.AP(tensor=osb.tensor, offset=osb.offset,
                                ap=[[osp, 128], [NB, 3], [1, NB]]))


# ----------------------------------------------------------------------------
# host side
# ----------------------------------------------------------------------------

_NC_CACHE = {}


def _get_nc(n_stages=4):
    if n_stages not in _NC_CACHE:
        _NC_CACHE[n_stages] = build_bass(n_stages)
    return _NC_CACHE[n_stages]


def _pack_xs1(src_half):
    xs = np.zeros((128, 2, DIM), np.float32)
    for slot in range(2):
        for r in range(8):
            row = slot * 8 + r
            if row >= ROWS_IN[0]:
                break
            xs[r * WBLK[0]:r * WBLK[0] + W_IN[0], slot, :] = src_half[:, row, :].T
    return xs


def _core_inputs(source, guidance, weights, core):
    b, half = core // 2, core % 2
    flip = (half == 1)
    src = source[b]
    gd = guidance[b]
    if flip:
        src = src[:, ::-1, :]
        gd = gd[:, ::-1, :]
    src_half = np.ascontiguousarray(src[:, 0:ROWS_IN[0], :]).astype(np.float32)
    perm = _perm_for_core(flip)

    ins = {
        "src_conv": src_half.reshape(DIM, -1),
        "xs1": _pack_xs1(src_half),
        "guid_hr": np.ascontiguousarray(gd[:, 0:HR, :]).reshape(3, -1).astype(np.float32),
        "ones1": np.ones((1, NB), np.float32),
    }
    parts = {"ident": np.eye(128, dtype=np.float32)}
    sel96 = np.zeros((96, 4), np.float32)
    bc96 = np.zeros((4, 96), np.float32)
    for di in range(3):
        for dj in range(3):
            for pq in range(4):
                sel96[di * 32 + dj * 4 + pq, pq] = 1.0
                bc96[pq, di * 32 + dj * 4 + pq] = 1.0
    sel36 = np.zeros((128, 4), np.float32)
    bc36 = np.zeros((4, 128), np.float32)
    for pq in range(4):
        for dj in range(3):
            for di in range(3):
                sel36[pq * 32 + dj * 3 + di, pq] = 1.0
                bc36[pq, pq * 32 + dj * 3 + di] = 1.0
    parts["sel96"], parts["bc96"] = sel96, bc96
    parts["sel36"], parts["bc36"] = sel36, bc36
    pjT = np.asarray(weights["proj_w"])[:, :, 0, 0].T.astype(np.float32)  # [387, 384]
    for c in range(3):
        parts[f"pjT{c}"] = pjT[128 * c:128 * (c + 1)]
    parts["pjTb4"] = np.concatenate(
        [pjT[384:387], np.asarray(weights["proj_b"]).reshape(1, DIM)], axis=0
    ).astype(np.float32)
    rparts = {"rident": np.eye(128, dtype=np.float32)}
    for c in range(3):
        gT = np.ascontiguousarray(gd[c].T).astype(np.float32)
        rparts[f"gT{c}_0"] = gT[0:128]
        rparts[f"gT{c}_1"] = gT[128:224]
    for s in range(4):
        CmT = np.ascontiguousarray(resize_matrix(224, W_IN[s]).T)
        RmTT = np.ascontiguousarray(resize_matrix(224, W_IN[s])[0:ROWS_IN[s]].T)
        rparts[f"Cm{s}_0"], rparts[f"Cm{s}_1"] = CmT[0:128], CmT[128:224]
        rparts[f"RmT{s}_0"], rparts[f"RmT{s}_1"] = RmTT[0:128], RmTT[128:224]
    for s in range(4):
        name = f"up{s + 1}"
        cw = np.asarray(weights[name + "_cw"])[:, :, 0, 0]
        ew = np.asarray(weights[name + "_ew"])
        cwT = np.ascontiguousarray(cw.T).astype(np.float32)   # [387, 64]
        for c in range(3):
            parts[f"cwT{s}_{c}"] = cwT[128 * c:128 * (c + 1)]
        parts[f"cwT{s}_g"] = cwT[384:387]
        ewp = ew[perm]
        if flip:
            ewp = ewp[:, :, ::-1, :]
        ebp = np.asarray(weights[name + "_eb"])[perm]
        if s != 2:
            taps = np.zeros((9, COMP, 128), np.float32)
            eb_ = np.zeros((128, 1), np.float32)
            for t_di in range(3):
                for t_dj in range(3):
                    tap = ewp[:, :, t_di, t_dj].T   # [64, 36] ch' order
                    for di in range(3):
                        for dj in range(3):
                            for pq in range(4):
                                taps[t_di * 3 + t_dj][:, pq * 32 + dj * 3 + di] = \
                                    tap[:, di * 12 + dj * 4 + pq]
            for di in range(3):
                for dj in range(3):
                    for pq in range(4):
                        eb_[pq * 32 + dj * 3 + di, 0] = ebp[di * 12 + dj * 4 + pq]
        else:
            taps = np.zeros((9, COMP, 96), np.float32)
            eb_ = np.zeros((96, 1), np.float32)
            for t_di in range(3):
                for t_dj in range(3):
                    tap = ewp[:, :, t_di, t_dj].T   # [64, 36] ch' order
                    for chp in range(36):
                        di_, rem = divmod(chp, 12)
                        taps[t_di * 3 + t_dj][:, di_ * 32 + rem] = tap[:, chp]
            for di in range(3):
                eb_[32 * di:32 * di + 12, 0] = ebp[12 * di:12 * di + 12]
        for t in range(9):
            parts[f"ewT{s}_{t}"] = taps[t]
        ins[f"eb{s}"] = eb_
        ins[f"cb{s}"] = np.asarray(weights[name + "_cb"]).reshape(COMP, 1).astype(np.float32)
        if s >= 2:
            import ml_dtypes
            ins[f"imgz{s}"] = np.zeros((4, IMG_ROWS[s], NB), ml_dtypes.bfloat16)
        else:
            ins[f"imgz{s}"] = np.zeros((4, IMG_ROWS[s], NB), np.float32)

    def pack(spec, pieces):
        cols = sum(c for _, _, c in spec)
        blob = np.zeros((128, cols), np.float32)
        c0 = 0
        for nm, p, c in spec:
            a = pieces[nm]
            assert a.shape == (p, c), (nm, a.shape, (p, c))
            blob[0:p, c0:c0 + c] = a
            c0 += c
        return blob

    ins["cblob"] = pack(_cblob_spec(), parts)
    ins["rblob"] = pack(_rblob_spec(), rparts)
    return ins


def kernel(source, guidance, _trace=False, **w):
    source = np.asarray(source, dtype=np.float32)
    guidance = np.asarray(guidance, dtype=np.float32)
    nc = _get_nc(4)
    in_maps = [_core_inputs(source, guidance, w, core) for core in range(8)]
    try:
        res = run_bass_kernel_spmd(nc, in_maps, core_ids=list(range(8)), trace=_trace)
    except ModuleNotFoundError:
        res = run_bass_kernel_spmd(nc, in_maps, core_ids=list(range(8)))
    if _trace and res.exec_time_ns is not None:
        print(f"HW exec time: {res.exec_time_ns} ns", flush=True)
        if res.instructions_and_trace:
            print("trace:", res.instructions_and_trace[1], flush=True)
    full = np.zeros((4, DIM, 224, 224), np.float32)
    for core in range(8):
        b, half = core // 2, core % 2
        o = np.asarray(res.results[core]["out"], dtype=np.float32)
        if half == 0:
            full[b, :, 0:HR, :] = o
        else:
            full[b, :, HR:224, :] = o[:, ::-1, :]
    return full
